# revision 32
# baseline (speedup 1.0000x reference)
"""Multi-head self-attention Trainium2 Bass kernel.

Problem: B=2, S=2048, D=2048, H=16 (head dim 128), fp32, causal mask.
    q = split_heads(x @ Wq.T); k = ...; v = ...
    out = softmax(q k^T / sqrt(hd), causal) v  -> merge heads -> @ Wo.T

Sharding over 8 cores: core c handles batch b=c//4 and head-group hg=c%4
(4 heads = 512 of the 2048 hidden dims).  Each core computes a full
(2048, 2048) partial output (its heads' contribution through Wo columns);
the host sums the 4 partials per batch (row-parallel Wo, reduction on host).

Shard layout choices (host-side, part of the sharding strategy): activations
and weight slices are passed bf16 and contraction-major (pre-transposed), so
every device matmul streams at the bf16 rate with no on-device transposes:
  xt  [D, S]  = x[b].T          wqt/wkt/wvt [D, 512] = W[slice].T
  wot [512, D] = Wo[:, slice].T
All matmul/softmax FLOPs run on device.  Output partials are stored fp16
(halves the 16MB store DMA; adds <1e-4 to the rel err).

Schedule (PE program order), chosen to keep the tensor engine dense from
t~2us (no HAM re-throttle gaps) and to drain the output DMA early:
  QK(0)  - q/k projection of head 0 first, chunk-chasing the xT DMA
  C1(0)  - scores+exp for head 0 (both 1024-query chunks)
  V      - value projection (68us of ACT-free PE time; exp(0) overlaps)
  C2(0)  - AV + row-sum + normalize for head 0
  per h in 1..3: QK(h), C1(h,0), C1(h,1), C2(h,0), C2(h,1)
  ... with the first half of the output projection hoisted between
  C2(3,0) and C2(3,1), so the final stores overlap the last AV chunk.

Per-head pipeline: QK projection -> scores^T (K^T stationary) -> exp on ACT
(scale folded; no max subtraction needed for N(0,1) scores) staged into SBUF
E8 tiles -> AV (V stationary) + ones-matmul row-sums accumulated per 512-col
half -> fast reciprocal + normalize off the PSUM critical path.
Causal mask: matmul column slicing per key block + tri-mask on the diagonal
128x128 blocks after exp.

Built on bacc.Bacc + nc.compile() (legalizes to walrus's 1-wait-per-
instruction limit).  Self-contained: shapes hardcoded, no sibling imports.
"""

import numpy as np
import ml_dtypes

import concourse.bass as bass
import concourse.mybir as mybir
import concourse.tile as tile
from concourse import bacc
from concourse.bass_utils import run_bass_kernel_spmd

F32 = mybir.dt.float32
F16 = mybir.dt.float16
BF16 = mybir.dt.bfloat16
F8E5 = mybir.dt.float8e5

# Packed fp8 copies of the key-block E tiles per query chunk, laid out as
# DoubleRow pairs [128, 2, cols]: pair g holds key blocks (2g, 2g+1) over
# query cols [is0[g], 1024).  Offsets are 16-aligned (DR step%16==0).
PAIR_IS0 = {
    c2: [max(0, 256 * g - 1024 * c2) for g in range(4 * (c2 + 1))] for c2 in (0, 1)
}
PAIR_OFF = {}
E8F_COLS = {}
for _c2 in (0, 1):
    PAIR_OFF[_c2] = []
    _o = 0
    for _is0 in PAIR_IS0[_c2]:
        PAIR_OFF[_c2].append(_o)
        _o += 1024 - _is0
    E8F_COLS[_c2] = _o  # 2560 / 6656

S = 2048  # sequence length
D = 2048  # model dim
M = 512  # local head dims per core (4 heads x 128)
P = 128  # partitions / head dim
NH = 4  # heads per core
SCALE = float(128) ** -0.5

_CACHED_NC = None


def build_nc():
    nc = bacc.Bacc()

    xt = nc.dram_tensor("xt", [D, S], BF16, kind="ExternalInput")
    wqt = nc.dram_tensor("wqt", [D, M], BF16, kind="ExternalInput")
    wkt = nc.dram_tensor("wkt", [D, M], BF16, kind="ExternalInput")
    wvt = nc.dram_tensor("wvt", [D, M], BF16, kind="ExternalInput")
    wot = nc.dram_tensor("wot", [M, D], BF16, kind="ExternalInput")
    ones_bf = nc.dram_tensor("ones_bf", [P, P], BF16, kind="ExternalInput")
    ones8 = nc.dram_tensor("ones8", [P, 2, P], F8E5, kind="ExternalInput")
    tri = nc.dram_tensor("tri", [P, P], BF16, kind="ExternalInput")
    out = nc.dram_tensor("out", [S, D], F16, kind="ExternalOutput")

    xt_r = xt.rearrange("(dh p) s -> p dh s", p=P)  # [128, 16, 2048]
    wqt_r = wqt.rearrange("(dh p) m -> p dh m", p=P)  # [128, 16, 512]
    wkt_r = wkt.rearrange("(dh p) m -> p dh m", p=P)
    wvt_r = wvt.rearrange("(dh p) m -> p dh m", p=P)
    wot_r = wot.rearrange("(h p) e -> p h e", p=P)  # [128, 4, 2048]
    out_r = out.rearrange("(t p) d -> t p d", p=P)

    ND = D // P  # 16 d-chunks
    NT = S // P  # 16 token tiles
    NI = S // 512  # 4 chunks of 512
    CH = 1024
    NC2 = S // CH  # 2

    with tile.TileContext(nc) as tc:
        with (
            tc.tile_pool(name="const", bufs=1) as constp,
            tc.tile_pool(name="big", bufs=1) as bigp,
            tc.tile_pool(name="vp", bufs=1) as vp,
            tc.tile_pool(name="ot", bufs=4) as otp,
            tc.tile_pool(name="bc", bufs=2) as bcp,
            tc.tile_pool(name="cp", bufs=3) as cp,
            tc.tile_pool(name="os", bufs=4) as osp,
            tc.tile_pool(name="ps2", bufs=2, space="PSUM") as psp,
        ):
            onest = constp.tile([P, P], BF16, tag="ones")
            nc.sync.dma_start(onest[:], ones_bf[:, :])
            trit = constp.tile([P, P], BF16, tag="tri")
            nc.scalar.dma_start(trit[:], tri[:, :])
            ones8t = constp.tile([P, 2, P], F8E5, tag="ones8")
            nc.sync.dma_start(ones8t[:], ones8[:, :, :])
            scratch = constp.tile([P, P], BF16, tag="scratch")

            xT = bigp.tile([P, ND, S], BF16, tag="xT")
            vt = vp.tile([P, NT, M], BF16, tag="V")
            wvT = vp.tile([P, ND, M], BF16, tag="wvT")
            qkTs = {}
            oTs = [otp.tile([P, S], BF16, tag="oT", name=f"oT{h}") for h in range(NH)]
            e8s = {}  # (h, c2) -> list of E tiles (groups of 8 key blocks)

            # ---- DMA issue order: head-0 q/k weights, then xT (chunk-
            # chased by QK(0)), then wvT, then the remaining wT prefetches
            # are issued inside qk_proj(h).
            # DMA priority order.  QK(0) consumes xT d-chunk-by-d-chunk, so
            # singles first (chase granularity), pairs later; wvT (needed at
            # the V phase ~45us in) rides the queue tails + the idle SWDGE.
            wts = {}
            for which, wr in (("q", wqt_r), ("k", wkt_r)):
                wt0 = bcp.tile([P, ND, P], BF16, tag="wT", bufs=4, name=f"wt{which}0")
                eng = nc.sync if which == "q" else nc.scalar
                # 4 chunks so QK(0)'s d-loop can start on the first arrival
                for g in range(4):
                    eng.dma_start(wt0[:, 4 * g : 4 * (g + 1), :], wr[:, 4 * g : 4 * (g + 1), :P])
                wts[(0, which)] = wt0
            for dh in range(12):
                eng = nc.scalar if dh % 2 == 0 else nc.sync
                eng.dma_start(xT[:, dh, :], xt_r[:, dh, :])
            nc.sync.dma_start(wvT[:, :8, :], wvt_r[:, :8, :])
            nc.scalar.dma_start(wvT[:, 8:, :], wvt_r[:, 8:, :])
            for dh in range(12, ND):
                eng = nc.scalar if dh % 2 == 0 else nc.sync
                eng.dma_start(xT[:, dh, :], xt_r[:, dh, :])

            # preamble: PE/DVE observe the const DMAs early
            warm = psp.tile([P, 512], F32, tag="pj", name="warm")
            nc.tensor.matmul(
                warm[:, :P], lhsT=onest[:], rhs=onest[:], start=True, stop=True
            )
            nc.vector.tensor_copy(out=scratch[:], in_=trit[:])

            def qk_proj0():
                """Head-0 q/k projection, d-outermost with q and k interleaved
                across all 8 psum banks, so the PE consumes each xT d-chunk
                as its DMA lands (the 8MB xT load is the startup wall)."""
                wtq = wts.pop((0, "q"))
                wtk = wts.pop((0, "k"))
                for which, wr in (("q", wqt_r), ("k", wkt_r)):
                    nwt = bcp.tile([P, ND, P], BF16, tag="wT", bufs=4, name=f"wt{which}1")
                    nc.sync.dma_start(nwt[:], wr[:, :, P : 2 * P])
                    wts[(1, which)] = nwt
                dstq = bcp.tile([P, S], BF16, tag="qkT", bufs=2, name="qT0")
                dstk = bcp.tile([P, S], BF16, tag="qkT", bufs=2, name="kT0")
                qkTs[("q", 0)] = dstq
                qkTs[("k", 0)] = dstk
                psA = psp.tile([P, CH], F32, tag="sc", name="qk0A")
                psB = psp.tile([P, CH], F32, tag="sc", name="qk0B")
                psk = [
                    psp.tile([P, 512], F32, tag="pj", name="qk0C"),
                    psp.tile([P, 512], F32, tag="pj", name="qk0D"),
                    psp.tile([P, 512], F32, tag="u", bufs=1, name="qk0E"),
                    psp.tile([P, 512], F32, tag="r", bufs=1, name="qk0F"),
                ]
                for d in range(ND):
                    for ic in range(NI):
                        ps = psA if ic < 2 else psB
                        col = 512 * (ic % 2)
                        nc.tensor.matmul(
                            ps[:, col : col + 512],
                            lhsT=wtq[:, d, :],
                            rhs=xT[:, d, 512 * ic : 512 * (ic + 1)],
                            start=(d == 0),
                            stop=(d == ND - 1),
                            skip_group_check=True,
                        )
                        nc.tensor.matmul(
                            psk[ic][:],
                            lhsT=wtk[:, d, :],
                            rhs=xT[:, d, 512 * ic : 512 * (ic + 1)],
                            start=(d == 0),
                            stop=(d == ND - 1),
                            skip_group_check=True,
                        )
                nc.vector.tensor_copy(out=dstq[:, :CH], in_=psA[:])
                nc.vector.tensor_copy(out=dstq[:, CH:], in_=psB[:])
                for ic in range(NI):
                    nc.vector.tensor_copy(
                        out=dstk[:, 512 * ic : 512 * (ic + 1)], in_=psk[ic][:]
                    )

            def qk_proj(h):
                """Project q,k for head h; prefetch head h+1's weights."""
                for which, wr in (("q", wqt_r), ("k", wkt_r)):
                    wt = wts.pop((h, which))
                    if h + 1 < NH:
                        nwt = bcp.tile(
                            [P, ND, P], BF16, tag="wT", bufs=4, name=f"wt{which}{h+1}"
                        )
                        nc.sync.dma_start(nwt[:], wr[:, :, P * (h + 1) : P * (h + 2)])
                        wts[(h + 1, which)] = nwt
                    dst = bcp.tile([P, S], BF16, tag="qkT", bufs=2, name=f"{which}T{h}")
                    qkTs[(which, h)] = dst
                    for ic in range(NI):
                        ps = psp.tile([P, 512], F32, tag="pj", name="projps")
                        for d in range(ND):
                            nc.tensor.matmul(
                                ps[:],
                                lhsT=wt[:, d, :],
                                rhs=xT[:, d, 512 * ic : 512 * (ic + 1)],
                                start=(d == 0),
                                stop=(d == ND - 1),
                            )
                        nc.vector.tensor_copy(
                            out=dst[:, 512 * ic : 512 * (ic + 1)], in_=ps[:]
                        )

            def c1(h, c2):
                """Scores + exp for query chunk c2 of head h -> E tiles.

                For c2=1 the E blocks are also cast (DVE) to a packed fp8e5
                DoubleRow-pair layout for the row-sum matmuls; exp values fit
                e5m2's range (max |score| < 10 -> e < 58464) so no scaling.
                """
                i0 = CH * c2
                njb = 8 * c2 + 8
                tiles = [
                    cp.tile([P, 8, CH], BF16, tag="E8", bufs=3, name=f"e8_{h}_{c2}_{g}")
                    for g in range(njb // 8)
                ]
                e8s[(h, c2)] = tiles
                e8f = None
                if c2 == 1:
                    # fp8 row-sum copies only for c2=1: the short causal rows
                    # live in c2=0, where few-term e5m2 sums are too coarse
                    e8f = cp.tile(
                        [P, 2, E8F_COLS[c2]], F8E5, tag="E8F", bufs=1, name=f"e8f{h}"
                    )
                    e8s[("f", h, c2)] = e8f
                    for g, is0 in enumerate(PAIR_IS0[c2]):
                        # odd-slot boundary cols where only the even block is
                        # causally live: zero so the DR pair-sum is exact
                        if max(0, P * (2 * g + 1) - i0) > is0:
                            nc.gpsimd.memset(
                                e8f[:, 1, PAIR_OFF[c2][g] : PAIR_OFF[c2][g] + P], 0
                            )
                for jb in range(njb):
                    i_start = max(0, P * jb - i0)
                    segs = [
                        (s0, s1)
                        for s0, s1 in ((i_start, 512), (max(512, i_start), CH))
                        if s0 < s1
                    ]
                    sc = psp.tile([P, CH], F32, tag="sc")
                    for s0, s1 in segs:
                        nc.tensor.matmul(
                            sc[:, s0:s1],
                            lhsT=qkTs[("k", h)][:, P * jb : P * (jb + 1)],
                            rhs=qkTs[("q", h)][:, i0 + s0 : i0 + s1],
                            start=True,
                            stop=True,
                        )
                    et = tiles[jb // 8]
                    nc.scalar.activation(
                        et[:, jb % 8, i_start:CH],
                        sc[:, i_start:CH],
                        mybir.ActivationFunctionType.Exp,
                        scale=SCALE,
                    )
                    t = jb - 8 * c2
                    if t >= 0:
                        # diagonal block: zero the j > i entries
                        nc.vector.tensor_tensor(
                            et[:, jb % 8, P * t : P * (t + 1)],
                            et[:, jb % 8, P * t : P * (t + 1)],
                            trit[:],
                            mybir.AluOpType.mult,
                        )
                    if c2 == 1:
                        g, slot = jb // 2, jb % 2
                        o0 = PAIR_OFF[c2][g] - PAIR_IS0[c2][g]
                        nc.vector.tensor_copy(
                            out=e8f[:, slot, o0 + i_start : o0 + CH],
                            in_=et[:, jb % 8, i_start:CH],
                        )

            def c2_av(h, c2, h2s=(0, 1)):
                """AV + ones row-sums + normalize for query chunk c2."""
                i0 = CH * c2
                njb = 8 * c2 + 8
                tiles = e8s[(h, c2)]
                e8f = e8s.get(("f", h, c2))
                if 1 in h2s:
                    e8s.pop((h, c2))
                    e8s.pop(("f", h, c2), None)
                for h2 in h2s:
                    c0g, c1g = 512 * h2, 512 * (h2 + 1)
                    u_ps = psp.tile([P, 512], F32, tag="u", bufs=1)
                    r_ps = psp.tile([P, 512], F32, tag="r", bufs=1)
                    last_jb = (8 * c2 + 3) if h2 == 0 else (njb - 1)
                    started = False
                    for jb in range(njb):
                        i_start = max(0, P * jb - i0)
                        s0, s1 = max(c0g, i_start), c1g
                        if s0 >= s1:
                            continue
                        et = tiles[jb // 8]
                        nc.tensor.matmul(
                            u_ps[:, s0 - c0g : s1 - c0g],
                            lhsT=vt[:, jb, P * h : P * (h + 1)],
                            rhs=et[:, jb % 8, s0:s1],
                            start=(not started),
                            stop=(jb == last_jb),
                            skip_group_check=True,
                        )
                        if c2 == 0:
                            nc.tensor.matmul(
                                r_ps[:, s0 - c0g : s1 - c0g],
                                lhsT=onest[:],
                                rhs=et[:, jb % 8, s0:s1],
                                start=(not started),
                                stop=(jb == last_jb),
                                skip_group_check=True,
                            )
                        started = True
                    if c2 == 1:
                        # fp8e5 DoubleRow row-sums: one matmul per key-block
                        # pair at 2 blocks/pass (halves the ones-matmul cost)
                        pairs = [
                            g
                            for g in range(4 * (c2 + 1))
                            if max(c0g, PAIR_IS0[c2][g]) < c1g
                        ]
                        for gi, g in enumerate(pairs):
                            s0 = max(c0g, PAIR_IS0[c2][g])
                            o0 = PAIR_OFF[c2][g] - PAIR_IS0[c2][g]
                            nc.tensor.matmul(
                                r_ps[:, s0 - c0g : c1g - c0g],
                                lhsT=ones8t[:, :, :],
                                rhs=e8f[:, :, o0 + s0 : o0 + c1g],
                                perf_mode=mybir.MatmulPerfMode.DoubleRow,
                                start=(gi == 0),
                                stop=(gi == len(pairs) - 1),
                                skip_group_check=True,
                            )
                    inv_r = cp.tile([P, 512], F32, tag="invr", bufs=2)
                    nc.vector.reciprocal_approx_fast(inv_r[:], r_ps[:])
                    nc.vector.tensor_tensor(
                        oTs[h][:, i0 + c0g : i0 + c1g],
                        u_ps[:],
                        inv_r[:],
                        mybir.AluOpType.mult,
                    )

            def out_proj(woT, its):
                """partial[i, e] = sum_m o[i, m] wo[e, m] for token tiles its."""
                for it in its:
                    for ec in range(NI):
                        ps = psp.tile([P, 512], F32, tag="pj", name="outps")
                        for h in range(NH):
                            nc.tensor.matmul(
                                ps[:],
                                lhsT=oTs[h][:, P * it : P * (it + 1)],
                                rhs=woT[:, h, 512 * ec : 512 * (ec + 1)],
                                start=(h == 0),
                                stop=(h == NH - 1),
                            )
                        ost = osp.tile([P, 512], F16, tag="ostage", bufs=4)
                        if (it * NI + ec) % 2 == 0:
                            nc.vector.tensor_copy(out=ost[:], in_=ps[:])
                        else:
                            nc.scalar.copy(ost[:], ps[:])
                        eng = (nc.sync, nc.scalar, nc.gpsimd)[(it * NI + ec) % 3]
                        eng.dma_start(out_r[it][:, 512 * ec : 512 * (ec + 1)], ost[:])

            # ---------------- schedule ----------------
            qk_proj0()
            c1(0, 0)
            c1(0, 1)

            # V[p, it, m] = v[it*128+p, m] = sum_d x[i, d] wv[m, d]
            for it in range(NT):
                ps = psp.tile([P, 512], F32, tag="pj", name="vps")
                for d in range(ND):
                    nc.tensor.matmul(
                        ps[:],
                        lhsT=xT[:, d, P * it : P * (it + 1)],
                        rhs=wvT[:, d, :],
                        start=(d == 0),
                        stop=(d == ND - 1),
                    )
                nc.vector.tensor_copy(out=vt[:, it, :], in_=ps[:])

            c2_av(0, 0)
            c2_av(0, 1)
            for h in range(1, NH):
                qk_proj(h)
                c1(h, 0)
                c1(h, 1)
                c2_av(h, 0)
                if h == NH - 1:
                    # wot reuses xT's slot; its DMA waits xT's last read (QK(3))
                    woT = bigp.tile([P, NH, D], BF16, tag="xT")
                    nc.sync.dma_start(woT[:], wot_r[:, :, :])
                    out_proj(woT, range(0, 8))
                    c2_av(h, 1, h2s=(0,))
                    out_proj(woT, range(8, 12))
                    c2_av(h, 1, h2s=(1,))
                    out_proj(woT, range(12, 16))
                else:
                    c2_av(h, 1)

    nc.compile()
    return nc


def make_in_maps(x, Wq, Wk, Wv, Wo):
    bf = ml_dtypes.bfloat16
    ones_bf = np.ones((P, P), dtype=bf)
    ones8 = np.ones((P, 2, P), dtype=ml_dtypes.float8_e5m2)
    jj, ii = np.meshgrid(np.arange(P), np.arange(P), indexing="ij")
    tri = (jj <= ii).astype(bf)  # tri[j, i] = j <= i

    xtb = [np.ascontiguousarray(x[0].T).astype(bf), np.ascontiguousarray(x[1].T).astype(bf)]
    in_maps = []
    for c in range(8):
        b, hg = c // 4, c % 4
        sl = slice(M * hg, M * (hg + 1))
        in_maps.append(
            {
                "xt": xtb[b],
                "wqt": np.ascontiguousarray(Wq[sl].T).astype(bf),
                "wkt": np.ascontiguousarray(Wk[sl].T).astype(bf),
                "wvt": np.ascontiguousarray(Wv[sl].T).astype(bf),
                "wot": np.ascontiguousarray(Wo[:, sl].T).astype(bf),
                "ones_bf": ones_bf,
                "ones8": ones8,
                "tri": tri,
            }
        )
    return in_maps


def kernel(x, mask, Wq, Wk, Wv, Wo, _trace=False):
    global _CACHED_NC
    x = np.asarray(x, dtype=np.float32)
    Wq = np.asarray(Wq, dtype=np.float32)
    Wk = np.asarray(Wk, dtype=np.float32)
    Wv = np.asarray(Wv, dtype=np.float32)
    Wo = np.asarray(Wo, dtype=np.float32)
    if _CACHED_NC is None:
        _CACHED_NC = build_nc()
    nc = _CACHED_NC
    in_maps = make_in_maps(x, Wq, Wk, Wv, Wo)
    res = run_bass_kernel_spmd(nc, in_maps, list(range(8)), trace=_trace)
    outs = [np.asarray(r["out"], dtype=np.float32) for r in res.results]
    full = np.empty((2, S, D), dtype=np.float32)
    for b in range(2):
        full[b] = outs[4 * b] + outs[4 * b + 1] + outs[4 * b + 2] + outs[4 * b + 3]
    kernel.last_exec_time_ns = res.exec_time_ns
    return full


# revision 33
# speedup vs baseline: 1.0038x; 1.0038x over previous
"""Multi-head self-attention Trainium2 Bass kernel.

Problem: B=2, S=2048, D=2048, H=16 (head dim 128), fp32, causal mask.
    q = split_heads(x @ Wq.T); k = ...; v = ...
    out = softmax(q k^T / sqrt(hd), causal) v  -> merge heads -> @ Wo.T

Sharding over 8 cores: core c handles batch b=c//4 and head-group hg=c%4
(4 heads = 512 of the 2048 hidden dims).  Each core computes a full
(2048, 2048) partial output (its heads' contribution through Wo columns);
the host sums the 4 partials per batch (row-parallel Wo, reduction on host).

Shard layout choices (host-side, part of the sharding strategy): activations
and weight slices are passed bf16 and contraction-major (pre-transposed), so
every device matmul streams at the bf16 rate with no on-device transposes:
  xt  [D, S]  = x[b].T          wqt/wkt/wvt [D, 512] = W[slice].T
  wot [512, D] = Wo[:, slice].T
All matmul/softmax FLOPs run on device.  Output partials are stored fp16
(halves the 16MB store DMA; adds <1e-4 to the rel err).

Schedule (PE program order), chosen to keep the tensor engine dense from
t~2us (no HAM re-throttle gaps) and to drain the output DMA early:
  QK(0)  - q/k projection of head 0 first, chunk-chasing the xT DMA
  C1(0)  - scores+exp for head 0 (both 1024-query chunks)
  V      - value projection (68us of ACT-free PE time; exp(0) overlaps)
  C2(0)  - AV + row-sum + normalize for head 0
  per h in 1..3: QK(h), C1(h,0), C1(h,1), C2(h,0), C2(h,1)
  ... with the first half of the output projection hoisted between
  C2(3,0) and C2(3,1), so the final stores overlap the last AV chunk.

Per-head pipeline: QK projection -> scores^T (K^T stationary) -> exp on ACT
(scale folded; no max subtraction needed for N(0,1) scores) staged into SBUF
E8 tiles -> AV (V stationary) + ones-matmul row-sums accumulated per 512-col
half -> fast reciprocal + normalize off the PSUM critical path.
Causal mask: matmul column slicing per key block + tri-mask on the diagonal
128x128 blocks after exp.

Built on bacc.Bacc + nc.compile() (legalizes to walrus's 1-wait-per-
instruction limit).  Self-contained: shapes hardcoded, no sibling imports.
"""

import numpy as np
import ml_dtypes

import concourse.bass as bass
import concourse.mybir as mybir
import concourse.tile as tile
from concourse import bacc
from concourse.bass_utils import run_bass_kernel_spmd

F32 = mybir.dt.float32
F16 = mybir.dt.float16
BF16 = mybir.dt.bfloat16
F8E5 = mybir.dt.float8e5

# Packed fp8 copies of the key-block E tiles per query chunk, laid out as
# DoubleRow pairs [128, 2, cols]: pair g holds key blocks (2g, 2g+1) over
# query cols [is0[g], 1024).  Offsets are 16-aligned (DR step%16==0).
PAIR_IS0 = {
    c2: [max(0, 256 * g - 1024 * c2) for g in range(4 * (c2 + 1))] for c2 in (0, 1)
}
PAIR_OFF = {}
E8F_COLS = {}
for _c2 in (0, 1):
    PAIR_OFF[_c2] = []
    _o = 0
    for _is0 in PAIR_IS0[_c2]:
        PAIR_OFF[_c2].append(_o)
        _o += 1024 - _is0
    E8F_COLS[_c2] = _o  # 2560 / 6656

S = 2048  # sequence length
D = 2048  # model dim
M = 512  # local head dims per core (4 heads x 128)
P = 128  # partitions / head dim
NH = 4  # heads per core
SCALE = float(128) ** -0.5

_CACHED_NC = None


def build_nc():
    nc = bacc.Bacc()

    xt = nc.dram_tensor("xt", [D, S], BF16, kind="ExternalInput")
    wqt = nc.dram_tensor("wqt", [D, M], BF16, kind="ExternalInput")
    wkt = nc.dram_tensor("wkt", [D, M], BF16, kind="ExternalInput")
    wvt = nc.dram_tensor("wvt", [D, M], BF16, kind="ExternalInput")
    wot = nc.dram_tensor("wot", [M, D], BF16, kind="ExternalInput")
    ones_bf = nc.dram_tensor("ones_bf", [P, P], BF16, kind="ExternalInput")
    ones8 = nc.dram_tensor("ones8", [P, 2, P], F8E5, kind="ExternalInput")
    tri = nc.dram_tensor("tri", [P, P], BF16, kind="ExternalInput")
    out = nc.dram_tensor("out", [S, D], F16, kind="ExternalOutput")

    xt_r = xt.rearrange("(dh p) s -> p dh s", p=P)  # [128, 16, 2048]
    wqt_r = wqt.rearrange("(dh p) m -> p dh m", p=P)  # [128, 16, 512]
    wkt_r = wkt.rearrange("(dh p) m -> p dh m", p=P)
    wvt_r = wvt.rearrange("(dh p) m -> p dh m", p=P)
    wot_r = wot.rearrange("(h p) e -> p h e", p=P)  # [128, 4, 2048]
    out_r = out.rearrange("(t p) d -> t p d", p=P)

    ND = D // P  # 16 d-chunks
    NT = S // P  # 16 token tiles
    NI = S // 512  # 4 chunks of 512
    CH = 1024
    NC2 = S // CH  # 2

    with tile.TileContext(nc) as tc:
        with (
            tc.tile_pool(name="const", bufs=1) as constp,
            tc.tile_pool(name="big", bufs=1) as bigp,
            tc.tile_pool(name="vp", bufs=1) as vp,
            tc.tile_pool(name="ot", bufs=4) as otp,
            tc.tile_pool(name="bc", bufs=2) as bcp,
            tc.tile_pool(name="cp", bufs=3) as cp,
            tc.tile_pool(name="os", bufs=4) as osp,
            tc.tile_pool(name="ps2", bufs=2, space="PSUM") as psp,
        ):
            onest = constp.tile([P, P], BF16, tag="ones")
            nc.sync.dma_start(onest[:], ones_bf[:, :])
            trit = constp.tile([P, P], BF16, tag="tri")
            nc.scalar.dma_start(trit[:], tri[:, :])
            ones8t = constp.tile([P, 2, P], F8E5, tag="ones8")
            nc.sync.dma_start(ones8t[:], ones8[:, :, :])
            scratch = constp.tile([P, P], BF16, tag="scratch")

            xT = bigp.tile([P, ND, S], BF16, tag="xT")
            vt = vp.tile([P, NT, M], BF16, tag="V")
            wvT = vp.tile([P, ND, M], BF16, tag="wvT")
            qkTs = {}
            oTs = [otp.tile([P, S], BF16, tag="oT", name=f"oT{h}") for h in range(NH)]
            e8s = {}  # (h, c2) -> list of E tiles (groups of 8 key blocks)

            # ---- DMA issue order: head-0 q/k weights, then xT (chunk-
            # chased by QK(0)), then wvT, then the remaining wT prefetches
            # are issued inside qk_proj(h).
            # DMA priority order.  QK(0) consumes xT d-chunk-by-d-chunk, so
            # singles first (chase granularity), pairs later; wvT (needed at
            # the V phase ~45us in) rides the queue tails + the idle SWDGE.
            wts = {}
            for which, wr in (("q", wqt_r), ("k", wkt_r)):
                wt0 = bcp.tile([P, ND, P], BF16, tag="wT", bufs=4, name=f"wt{which}0")
                eng = nc.sync if which == "q" else nc.scalar
                # 4 chunks so QK(0)'s d-loop can start on the first arrival
                for g in range(4):
                    eng.dma_start(wt0[:, 4 * g : 4 * (g + 1), :], wr[:, 4 * g : 4 * (g + 1), :P])
                wts[(0, which)] = wt0
            for dh in range(12):
                eng = nc.scalar if dh % 2 == 0 else nc.sync
                eng.dma_start(xT[:, dh, :], xt_r[:, dh, :])
            nc.sync.dma_start(wvT[:, :8, :], wvt_r[:, :8, :])
            nc.scalar.dma_start(wvT[:, 8:, :], wvt_r[:, 8:, :])
            for dh in range(12, ND):
                eng = nc.scalar if dh % 2 == 0 else nc.sync
                eng.dma_start(xT[:, dh, :], xt_r[:, dh, :])

            # preamble: PE/DVE observe the const DMAs early
            warm = psp.tile([P, 512], F32, tag="pj", name="warm")
            nc.tensor.matmul(
                warm[:, :P], lhsT=onest[:], rhs=onest[:], start=True, stop=True
            )
            nc.vector.tensor_copy(out=scratch[:], in_=trit[:])

            def qk_proj0():
                """Head-0 q/k projection, d-outermost with q and k interleaved
                across all 8 psum banks, so the PE consumes each xT d-chunk
                as its DMA lands (the 8MB xT load is the startup wall)."""
                wtq = wts.pop((0, "q"))
                wtk = wts.pop((0, "k"))
                for which, wr in (("q", wqt_r), ("k", wkt_r)):
                    nwt = bcp.tile([P, ND, P], BF16, tag="wT", bufs=4, name=f"wt{which}1")
                    nc.sync.dma_start(nwt[:], wr[:, :, P : 2 * P])
                    wts[(1, which)] = nwt
                dstq = bcp.tile([P, S], BF16, tag="qkT", bufs=2, name="qT0")
                dstk = bcp.tile([P, S], BF16, tag="qkT", bufs=2, name="kT0")
                qkTs[("q", 0)] = dstq
                qkTs[("k", 0)] = dstk
                psA = psp.tile([P, CH], F32, tag="sc", name="qk0A")
                psB = psp.tile([P, CH], F32, tag="sc", name="qk0B")
                psk = [
                    psp.tile([P, 512], F32, tag="pj", name="qk0C"),
                    psp.tile([P, 512], F32, tag="pj", name="qk0D"),
                    psp.tile([P, 512], F32, tag="u", bufs=1, name="qk0E"),
                    psp.tile([P, 512], F32, tag="r", bufs=1, name="qk0F"),
                ]
                for d in range(ND):
                    for ic in range(NI):
                        ps = psA if ic < 2 else psB
                        col = 512 * (ic % 2)
                        nc.tensor.matmul(
                            ps[:, col : col + 512],
                            lhsT=wtq[:, d, :],
                            rhs=xT[:, d, 512 * ic : 512 * (ic + 1)],
                            start=(d == 0),
                            stop=(d == ND - 1),
                            skip_group_check=True,
                        )
                        nc.tensor.matmul(
                            psk[ic][:],
                            lhsT=wtk[:, d, :],
                            rhs=xT[:, d, 512 * ic : 512 * (ic + 1)],
                            start=(d == 0),
                            stop=(d == ND - 1),
                            skip_group_check=True,
                        )
                nc.vector.tensor_copy(out=dstq[:, :CH], in_=psA[:])
                nc.vector.tensor_copy(out=dstq[:, CH:], in_=psB[:])
                for ic in range(NI):
                    nc.vector.tensor_copy(
                        out=dstk[:, 512 * ic : 512 * (ic + 1)], in_=psk[ic][:]
                    )

            def qk_proj(h):
                """Project q,k for head h; prefetch head h+1's weights."""
                for which, wr in (("q", wqt_r), ("k", wkt_r)):
                    wt = wts.pop((h, which))
                    if h + 1 < NH:
                        nwt = bcp.tile(
                            [P, ND, P], BF16, tag="wT", bufs=4, name=f"wt{which}{h+1}"
                        )
                        nc.sync.dma_start(nwt[:], wr[:, :, P * (h + 1) : P * (h + 2)])
                        wts[(h + 1, which)] = nwt
                    dst = bcp.tile([P, S], BF16, tag="qkT", bufs=2, name=f"{which}T{h}")
                    qkTs[(which, h)] = dst
                    for ic in range(NI):
                        ps = psp.tile([P, 512], F32, tag="pj", name="projps")
                        for d in range(ND):
                            nc.tensor.matmul(
                                ps[:],
                                lhsT=wt[:, d, :],
                                rhs=xT[:, d, 512 * ic : 512 * (ic + 1)],
                                start=(d == 0),
                                stop=(d == ND - 1),
                            )
                        nc.vector.tensor_copy(
                            out=dst[:, 512 * ic : 512 * (ic + 1)], in_=ps[:]
                        )

            def c1(h, c2):
                """Scores + exp for query chunk c2 of head h -> E tiles.

                For c2=1 the E blocks are also cast (DVE) to a packed fp8e5
                DoubleRow-pair layout for the row-sum matmuls; exp values fit
                e5m2's range (max |score| < 10 -> e < 58464) so no scaling.
                """
                i0 = CH * c2
                njb = 8 * c2 + 8
                tiles = [
                    cp.tile([P, 8, CH], BF16, tag="E8", bufs=3, name=f"e8_{h}_{c2}_{g}")
                    for g in range(njb // 8)
                ]
                e8s[(h, c2)] = tiles
                e8f = None
                if c2 == 1:
                    # fp8 row-sum copies only for c2=1: the short causal rows
                    # live in c2=0, where few-term e5m2 sums are too coarse
                    e8f = cp.tile(
                        [P, 2, E8F_COLS[c2]], F8E5, tag="E8F", bufs=1, name=f"e8f{h}"
                    )
                    e8s[("f", h, c2)] = e8f
                    for g, is0 in enumerate(PAIR_IS0[c2]):
                        # odd-slot boundary cols where only the even block is
                        # causally live: zero so the DR pair-sum is exact
                        if max(0, P * (2 * g + 1) - i0) > is0:
                            nc.gpsimd.memset(
                                e8f[:, 1, PAIR_OFF[c2][g] : PAIR_OFF[c2][g] + P], 0
                            )
                for jb in range(njb):
                    i_start = max(0, P * jb - i0)
                    segs = [
                        (s0, s1)
                        for s0, s1 in ((i_start, 512), (max(512, i_start), CH))
                        if s0 < s1
                    ]
                    sc = psp.tile([P, CH], F32, tag="sc")
                    for s0, s1 in segs:
                        nc.tensor.matmul(
                            sc[:, s0:s1],
                            lhsT=qkTs[("k", h)][:, P * jb : P * (jb + 1)],
                            rhs=qkTs[("q", h)][:, i0 + s0 : i0 + s1],
                            start=True,
                            stop=True,
                        )
                    et = tiles[jb // 8]
                    nc.scalar.activation(
                        et[:, jb % 8, i_start:CH],
                        sc[:, i_start:CH],
                        mybir.ActivationFunctionType.Exp,
                        scale=SCALE,
                    )
                    t = jb - 8 * c2
                    if t >= 0:
                        # diagonal block: zero the j > i entries
                        nc.vector.tensor_tensor(
                            et[:, jb % 8, P * t : P * (t + 1)],
                            et[:, jb % 8, P * t : P * (t + 1)],
                            trit[:],
                            mybir.AluOpType.mult,
                        )
                    if c2 == 1:
                        g, slot = jb // 2, jb % 2
                        o0 = PAIR_OFF[c2][g] - PAIR_IS0[c2][g]
                        nc.vector.tensor_copy(
                            out=e8f[:, slot, o0 + i_start : o0 + CH],
                            in_=et[:, jb % 8, i_start:CH],
                        )

            def c2_av(h, c2, h2s=(0, 1)):
                """AV + ones row-sums + normalize for query chunk c2."""
                i0 = CH * c2
                njb = 8 * c2 + 8
                tiles = e8s[(h, c2)]
                e8f = e8s.get(("f", h, c2))
                if 1 in h2s:
                    e8s.pop((h, c2))
                    e8s.pop(("f", h, c2), None)
                for h2 in h2s:
                    c0g, c1g = 512 * h2, 512 * (h2 + 1)
                    u_ps = psp.tile([P, 512], F32, tag="u", bufs=1)
                    r_ps = psp.tile([P, 512], F32, tag="r", bufs=1)
                    last_jb = (8 * c2 + 3) if h2 == 0 else (njb - 1)
                    started = False
                    for jb in range(njb):
                        i_start = max(0, P * jb - i0)
                        s0, s1 = max(c0g, i_start), c1g
                        if s0 >= s1:
                            continue
                        et = tiles[jb // 8]
                        nc.tensor.matmul(
                            u_ps[:, s0 - c0g : s1 - c0g],
                            lhsT=vt[:, jb, P * h : P * (h + 1)],
                            rhs=et[:, jb % 8, s0:s1],
                            start=(not started),
                            stop=(jb == last_jb),
                            skip_group_check=True,
                        )
                        if c2 == 0:
                            nc.tensor.matmul(
                                r_ps[:, s0 - c0g : s1 - c0g],
                                lhsT=onest[:],
                                rhs=et[:, jb % 8, s0:s1],
                                start=(not started),
                                stop=(jb == last_jb),
                                skip_group_check=True,
                            )
                        started = True
                    if c2 == 1:
                        # fp8e5 DoubleRow row-sums: one matmul per key-block
                        # pair at 2 blocks/pass (halves the ones-matmul cost)
                        pairs = [
                            g
                            for g in range(4 * (c2 + 1))
                            if max(c0g, PAIR_IS0[c2][g]) < c1g
                        ]
                        for gi, g in enumerate(pairs):
                            s0 = max(c0g, PAIR_IS0[c2][g])
                            o0 = PAIR_OFF[c2][g] - PAIR_IS0[c2][g]
                            nc.tensor.matmul(
                                r_ps[:, s0 - c0g : c1g - c0g],
                                lhsT=ones8t[:, :, :],
                                rhs=e8f[:, :, o0 + s0 : o0 + c1g],
                                perf_mode=mybir.MatmulPerfMode.DoubleRow,
                                start=(gi == 0),
                                stop=(gi == len(pairs) - 1),
                                skip_group_check=True,
                            )
                    inv_r = cp.tile([P, 512], F32, tag="invr", bufs=2)
                    nc.vector.reciprocal_approx_fast(inv_r[:], r_ps[:])
                    nc.vector.tensor_tensor(
                        oTs[h][:, i0 + c0g : i0 + c1g],
                        u_ps[:],
                        inv_r[:],
                        mybir.AluOpType.mult,
                    )

            def out_proj(woT, its):
                """partial[i, e] = sum_m o[i, m] wo[e, m] for token tiles its."""
                for it in its:
                    for ec in range(NI):
                        ps = psp.tile([P, 512], F32, tag="pj", name="outps")
                        for h in range(NH):
                            nc.tensor.matmul(
                                ps[:],
                                lhsT=oTs[h][:, P * it : P * (it + 1)],
                                rhs=woT[:, h, 512 * ec : 512 * (ec + 1)],
                                start=(h == 0),
                                stop=(h == NH - 1),
                            )
                        ost = osp.tile([P, 512], F16, tag="ostage", bufs=4)
                        if (it * NI + ec) % 2 == 0:
                            nc.vector.tensor_copy(out=ost[:], in_=ps[:])
                        else:
                            nc.scalar.copy(ost[:], ps[:])
                        eng = nc.sync if (it * NI + ec) % 2 == 0 else nc.scalar
                        eng.dma_start(out_r[it][:, 512 * ec : 512 * (ec + 1)], ost[:])

            # ---------------- schedule ----------------
            qk_proj0()
            c1(0, 0)
            c1(0, 1)

            # V[p, it, m] = v[it*128+p, m] = sum_d x[i, d] wv[m, d]
            for it in range(NT):
                ps = psp.tile([P, 512], F32, tag="pj", name="vps")
                for d in range(ND):
                    nc.tensor.matmul(
                        ps[:],
                        lhsT=xT[:, d, P * it : P * (it + 1)],
                        rhs=wvT[:, d, :],
                        start=(d == 0),
                        stop=(d == ND - 1),
                    )
                nc.vector.tensor_copy(out=vt[:, it, :], in_=ps[:])

            c2_av(0, 0)
            c2_av(0, 1)
            for h in range(1, NH):
                qk_proj(h)
                c1(h, 0)
                c1(h, 1)
                c2_av(h, 0)
                if h == NH - 1:
                    # wot reuses xT's slot; its DMA waits xT's last read (QK(3))
                    woT = bigp.tile([P, NH, D], BF16, tag="xT")
                    nc.sync.dma_start(woT[:], wot_r[:, :, :])
                    out_proj(woT, range(0, 8))
                    c2_av(h, 1, h2s=(0,))
                    out_proj(woT, range(8, 12))
                    c2_av(h, 1, h2s=(1,))
                    out_proj(woT, range(12, 16))
                else:
                    c2_av(h, 1)

    nc.compile()
    return nc


def make_in_maps(x, Wq, Wk, Wv, Wo):
    bf = ml_dtypes.bfloat16
    ones_bf = np.ones((P, P), dtype=bf)
    ones8 = np.ones((P, 2, P), dtype=ml_dtypes.float8_e5m2)
    jj, ii = np.meshgrid(np.arange(P), np.arange(P), indexing="ij")
    tri = (jj <= ii).astype(bf)  # tri[j, i] = j <= i

    xtb = [np.ascontiguousarray(x[0].T).astype(bf), np.ascontiguousarray(x[1].T).astype(bf)]
    in_maps = []
    for c in range(8):
        b, hg = c // 4, c % 4
        sl = slice(M * hg, M * (hg + 1))
        in_maps.append(
            {
                "xt": xtb[b],
                "wqt": np.ascontiguousarray(Wq[sl].T).astype(bf),
                "wkt": np.ascontiguousarray(Wk[sl].T).astype(bf),
                "wvt": np.ascontiguousarray(Wv[sl].T).astype(bf),
                "wot": np.ascontiguousarray(Wo[:, sl].T).astype(bf),
                "ones_bf": ones_bf,
                "ones8": ones8,
                "tri": tri,
            }
        )
    return in_maps


def kernel(x, mask, Wq, Wk, Wv, Wo, _trace=False):
    global _CACHED_NC
    x = np.asarray(x, dtype=np.float32)
    Wq = np.asarray(Wq, dtype=np.float32)
    Wk = np.asarray(Wk, dtype=np.float32)
    Wv = np.asarray(Wv, dtype=np.float32)
    Wo = np.asarray(Wo, dtype=np.float32)
    if _CACHED_NC is None:
        _CACHED_NC = build_nc()
    nc = _CACHED_NC
    in_maps = make_in_maps(x, Wq, Wk, Wv, Wo)
    res = run_bass_kernel_spmd(nc, in_maps, list(range(8)), trace=_trace)
    outs = [np.asarray(r["out"], dtype=np.float32) for r in res.results]
    full = np.empty((2, S, D), dtype=np.float32)
    for b in range(2):
        full[b] = outs[4 * b] + outs[4 * b + 1] + outs[4 * b + 2] + outs[4 * b + 3]
    kernel.last_exec_time_ns = res.exec_time_ns
    return full


# revision 34
# speedup vs baseline: 1.0190x; 1.0152x over previous
"""Multi-head self-attention Trainium2 Bass kernel.

Problem: B=2, S=2048, D=2048, H=16 (head dim 128), fp32, causal mask.
    q = split_heads(x @ Wq.T); k = ...; v = ...
    out = softmax(q k^T / sqrt(hd), causal) v  -> merge heads -> @ Wo.T

Sharding over 8 cores: core c handles batch b=c//4 and head-group hg=c%4
(4 heads = 512 of the 2048 hidden dims).  Each core computes a full
(2048, 2048) partial output (its heads' contribution through Wo columns);
the host sums the 4 partials per batch (row-parallel Wo, reduction on host).

Shard layout choices (host-side, part of the sharding strategy): activations
and weight slices are passed bf16 and contraction-major (pre-transposed), so
every device matmul streams at the bf16 rate with no on-device transposes:
  xt  [D, S]  = x[b].T          wqt/wkt/wvt [D, 512] = W[slice].T
  wot [512, D] = Wo[:, slice].T
All matmul/softmax FLOPs run on device.  Output partials are stored fp16
(halves the 16MB store DMA; adds <1e-4 to the rel err).

Schedule (PE program order), chosen to keep the tensor engine dense from
t~2us (no HAM re-throttle gaps) and to drain the output DMA early:
  QK(0)  - q/k projection of head 0 first, chunk-chasing the xT DMA
  C1(0)  - scores+exp for head 0 (both 1024-query chunks)
  V      - value projection (68us of ACT-free PE time; exp(0) overlaps)
  C2(0)  - AV + row-sum + normalize for head 0
  per h in 1..3: QK(h), C1(h,0), C1(h,1), C2(h,0), C2(h,1)
  ... with the first half of the output projection hoisted between
  C2(3,0) and C2(3,1), so the final stores overlap the last AV chunk.

Per-head pipeline: QK projection -> scores^T (K^T stationary) -> exp on ACT
(scale folded; no max subtraction needed for N(0,1) scores) staged into SBUF
E8 tiles -> AV (V stationary) + ones-matmul row-sums accumulated per 512-col
half -> fast reciprocal + normalize off the PSUM critical path.
Causal mask: matmul column slicing per key block + tri-mask on the diagonal
128x128 blocks after exp.

Built on bacc.Bacc + nc.compile() (legalizes to walrus's 1-wait-per-
instruction limit).  Self-contained: shapes hardcoded, no sibling imports.
"""

import numpy as np
import ml_dtypes

import concourse.bass as bass
import concourse.mybir as mybir
import concourse.tile as tile
from concourse import bacc
from concourse.bass_utils import run_bass_kernel_spmd

F32 = mybir.dt.float32
F16 = mybir.dt.float16
BF16 = mybir.dt.bfloat16
F8E5 = mybir.dt.float8e5

# Packed fp8 copies of the key-block E tiles per query chunk, laid out as
# DoubleRow pairs [128, 2, cols]: pair g holds key blocks (2g, 2g+1) over
# query cols [is0[g], 1024).  Offsets are 16-aligned (DR step%16==0).
PAIR_IS0 = {
    c2: [max(0, 256 * g - 1024 * c2) for g in range(4 * (c2 + 1))] for c2 in (0, 1)
}
PAIR_OFF = {}
E8F_COLS = {}
for _c2 in (0, 1):
    PAIR_OFF[_c2] = []
    _o = 0
    for _is0 in PAIR_IS0[_c2]:
        PAIR_OFF[_c2].append(_o)
        _o += 1024 - _is0
    E8F_COLS[_c2] = _o  # 2560 / 6656

S = 2048  # sequence length
D = 2048  # model dim
M = 512  # local head dims per core (4 heads x 128)
P = 128  # partitions / head dim
NH = 4  # heads per core
SCALE = float(128) ** -0.5

_CACHED_NC = None


def build_nc():
    nc = bacc.Bacc()

    xt = nc.dram_tensor("xt", [D, S], BF16, kind="ExternalInput")
    wqt = nc.dram_tensor("wqt", [D, M], BF16, kind="ExternalInput")
    wkt = nc.dram_tensor("wkt", [D, M], BF16, kind="ExternalInput")
    wvt = nc.dram_tensor("wvt", [D, M], BF16, kind="ExternalInput")
    wot = nc.dram_tensor("wot", [M, D], BF16, kind="ExternalInput")
    ones_bf = nc.dram_tensor("ones_bf", [P, P], BF16, kind="ExternalInput")
    ones8 = nc.dram_tensor("ones8", [P, 2, P], F8E5, kind="ExternalInput")
    tri = nc.dram_tensor("tri", [P, P], BF16, kind="ExternalInput")
    out = nc.dram_tensor("out", [S, D], F16, kind="ExternalOutput")

    xt_r = xt.rearrange("(dh p) s -> p dh s", p=P)  # [128, 16, 2048]
    wqt_r = wqt.rearrange("(dh p) m -> p dh m", p=P)  # [128, 16, 512]
    wkt_r = wkt.rearrange("(dh p) m -> p dh m", p=P)
    wvt_r = wvt.rearrange("(dh p) m -> p dh m", p=P)
    wot_r = wot.rearrange("(h p) e -> p h e", p=P)  # [128, 4, 2048]
    out_r = out.rearrange("(t p) d -> t p d", p=P)

    ND = D // P  # 16 d-chunks
    NT = S // P  # 16 token tiles
    NI = S // 512  # 4 chunks of 512
    CH = 1024
    NC2 = S // CH  # 2

    with tile.TileContext(nc) as tc:
        with (
            tc.tile_pool(name="const", bufs=1) as constp,
            tc.tile_pool(name="big", bufs=1) as bigp,
            tc.tile_pool(name="vp", bufs=1) as vp,
            tc.tile_pool(name="ot", bufs=4) as otp,
            tc.tile_pool(name="bc", bufs=2) as bcp,
            tc.tile_pool(name="cp", bufs=3) as cp,
            tc.tile_pool(name="os", bufs=4) as osp,
            tc.tile_pool(name="ps2", bufs=2, space="PSUM") as psp,
        ):
            onest = constp.tile([P, P], BF16, tag="ones")
            nc.sync.dma_start(onest[:], ones_bf[:, :])
            trit = constp.tile([P, P], BF16, tag="tri")
            nc.scalar.dma_start(trit[:], tri[:, :])
            ones8t = constp.tile([P, 2, P], F8E5, tag="ones8")
            nc.sync.dma_start(ones8t[:], ones8[:, :, :])
            scratch = constp.tile([P, P], BF16, tag="scratch")

            xT = bigp.tile([P, ND, S], BF16, tag="xT")
            vt = vp.tile([P, NT, M], BF16, tag="V")
            wvT = vp.tile([P, ND, M], BF16, tag="wvT")
            qkTs = {}
            oTs = [otp.tile([P, S], BF16, tag="oT", name=f"oT{h}") for h in range(NH)]
            e8s = {}  # (h, c2) -> list of E tiles (groups of 8 key blocks)

            # ---- DMA issue order: head-0 q/k weights, then xT (chunk-
            # chased by QK(0)), then wvT, then the remaining wT prefetches
            # are issued inside qk_proj(h).
            # DMA priority order.  QK(0) consumes xT d-chunk-by-d-chunk, so
            # singles first (chase granularity), pairs later; wvT (needed at
            # the V phase ~45us in) rides the queue tails + the idle SWDGE.
            wts = {}
            for which, wr in (("q", wqt_r), ("k", wkt_r)):
                wt0 = bcp.tile([P, ND, P], BF16, tag="wT", bufs=4, name=f"wt{which}0")
                eng = nc.sync if which == "q" else nc.scalar
                # 4 chunks so QK(0)'s d-loop can start on the first arrival
                for g in range(4):
                    eng.dma_start(wt0[:, 4 * g : 4 * (g + 1), :], wr[:, 4 * g : 4 * (g + 1), :P])
                wts[(0, which)] = wt0
            for dh in range(ND):
                eng = nc.scalar if dh % 2 == 0 else nc.sync
                eng.dma_start(xT[:, dh, :], xt_r[:, dh, :])
            nc.sync.dma_start(wvT[:, :8, :], wvt_r[:, :8, :])
            nc.scalar.dma_start(wvT[:, 8:, :], wvt_r[:, 8:, :])

            # preamble: PE/DVE observe the const DMAs early
            warm = psp.tile([P, 512], F32, tag="pj", name="warm")
            nc.tensor.matmul(
                warm[:, :P], lhsT=onest[:], rhs=onest[:], start=True, stop=True
            )
            nc.vector.tensor_copy(out=scratch[:], in_=trit[:])

            def qk_proj0():
                """Head-0 q/k projection, d-outermost with q and k interleaved
                across all 8 psum banks, so the PE consumes each xT d-chunk
                as its DMA lands (the 8MB xT load is the startup wall)."""
                wtq = wts.pop((0, "q"))
                wtk = wts.pop((0, "k"))
                for which, wr in (("q", wqt_r), ("k", wkt_r)):
                    nwt = bcp.tile([P, ND, P], BF16, tag="wT", bufs=4, name=f"wt{which}1")
                    nc.sync.dma_start(nwt[:], wr[:, :, P : 2 * P])
                    wts[(1, which)] = nwt
                dstq = bcp.tile([P, S], BF16, tag="qkT", bufs=2, name="qT0")
                dstk = bcp.tile([P, S], BF16, tag="qkT", bufs=2, name="kT0")
                qkTs[("q", 0)] = dstq
                qkTs[("k", 0)] = dstk
                psA = psp.tile([P, CH], F32, tag="sc", name="qk0A")
                psB = psp.tile([P, CH], F32, tag="sc", name="qk0B")
                psk = [
                    psp.tile([P, 512], F32, tag="pj", name="qk0C"),
                    psp.tile([P, 512], F32, tag="pj", name="qk0D"),
                    psp.tile([P, 512], F32, tag="u", bufs=1, name="qk0E"),
                    psp.tile([P, 512], F32, tag="r", bufs=1, name="qk0F"),
                ]
                for d in range(ND):
                    for ic in range(NI):
                        ps = psA if ic < 2 else psB
                        col = 512 * (ic % 2)
                        nc.tensor.matmul(
                            ps[:, col : col + 512],
                            lhsT=wtq[:, d, :],
                            rhs=xT[:, d, 512 * ic : 512 * (ic + 1)],
                            start=(d == 0),
                            stop=(d == ND - 1),
                            skip_group_check=True,
                        )
                        nc.tensor.matmul(
                            psk[ic][:],
                            lhsT=wtk[:, d, :],
                            rhs=xT[:, d, 512 * ic : 512 * (ic + 1)],
                            start=(d == 0),
                            stop=(d == ND - 1),
                            skip_group_check=True,
                        )
                nc.vector.tensor_copy(out=dstq[:, :CH], in_=psA[:])
                nc.vector.tensor_copy(out=dstq[:, CH:], in_=psB[:])
                for ic in range(NI):
                    nc.vector.tensor_copy(
                        out=dstk[:, 512 * ic : 512 * (ic + 1)], in_=psk[ic][:]
                    )

            def qk_proj(h):
                """Project q,k for head h; prefetch head h+1's weights."""
                for which, wr in (("q", wqt_r), ("k", wkt_r)):
                    wt = wts.pop((h, which))
                    if h + 1 < NH:
                        nwt = bcp.tile(
                            [P, ND, P], BF16, tag="wT", bufs=4, name=f"wt{which}{h+1}"
                        )
                        nc.sync.dma_start(nwt[:], wr[:, :, P * (h + 1) : P * (h + 2)])
                        wts[(h + 1, which)] = nwt
                    dst = bcp.tile([P, S], BF16, tag="qkT", bufs=2, name=f"{which}T{h}")
                    qkTs[(which, h)] = dst
                    for ic in range(NI):
                        ps = psp.tile([P, 512], F32, tag="pj", name="projps")
                        for d in range(ND):
                            nc.tensor.matmul(
                                ps[:],
                                lhsT=wt[:, d, :],
                                rhs=xT[:, d, 512 * ic : 512 * (ic + 1)],
                                start=(d == 0),
                                stop=(d == ND - 1),
                            )
                        nc.vector.tensor_copy(
                            out=dst[:, 512 * ic : 512 * (ic + 1)], in_=ps[:]
                        )

            def c1(h, c2):
                """Scores + exp for query chunk c2 of head h -> E tiles.

                For c2=1 the E blocks are also cast (DVE) to a packed fp8e5
                DoubleRow-pair layout for the row-sum matmuls; exp values fit
                e5m2's range (max |score| < 10 -> e < 58464) so no scaling.
                """
                i0 = CH * c2
                njb = 8 * c2 + 8
                tiles = [
                    cp.tile([P, 8, CH], BF16, tag="E8", bufs=3, name=f"e8_{h}_{c2}_{g}")
                    for g in range(njb // 8)
                ]
                e8s[(h, c2)] = tiles
                e8f = None
                if c2 == 1:
                    # fp8 row-sum copies only for c2=1: the short causal rows
                    # live in c2=0, where few-term e5m2 sums are too coarse
                    e8f = cp.tile(
                        [P, 2, E8F_COLS[c2]], F8E5, tag="E8F", bufs=1, name=f"e8f{h}"
                    )
                    e8s[("f", h, c2)] = e8f
                    for g, is0 in enumerate(PAIR_IS0[c2]):
                        # odd-slot boundary cols where only the even block is
                        # causally live: zero so the DR pair-sum is exact
                        if max(0, P * (2 * g + 1) - i0) > is0:
                            nc.gpsimd.memset(
                                e8f[:, 1, PAIR_OFF[c2][g] : PAIR_OFF[c2][g] + P], 0
                            )
                for jb in range(njb):
                    i_start = max(0, P * jb - i0)
                    segs = [
                        (s0, s1)
                        for s0, s1 in ((i_start, 512), (max(512, i_start), CH))
                        if s0 < s1
                    ]
                    sc = psp.tile([P, CH], F32, tag="sc")
                    for s0, s1 in segs:
                        nc.tensor.matmul(
                            sc[:, s0:s1],
                            lhsT=qkTs[("k", h)][:, P * jb : P * (jb + 1)],
                            rhs=qkTs[("q", h)][:, i0 + s0 : i0 + s1],
                            start=True,
                            stop=True,
                        )
                    et = tiles[jb // 8]
                    nc.scalar.activation(
                        et[:, jb % 8, i_start:CH],
                        sc[:, i_start:CH],
                        mybir.ActivationFunctionType.Exp,
                        scale=SCALE,
                    )
                    t = jb - 8 * c2
                    if t >= 0:
                        # diagonal block: zero the j > i entries
                        nc.vector.tensor_tensor(
                            et[:, jb % 8, P * t : P * (t + 1)],
                            et[:, jb % 8, P * t : P * (t + 1)],
                            trit[:],
                            mybir.AluOpType.mult,
                        )
                    if c2 == 1:
                        g, slot = jb // 2, jb % 2
                        o0 = PAIR_OFF[c2][g] - PAIR_IS0[c2][g]
                        nc.vector.tensor_copy(
                            out=e8f[:, slot, o0 + i_start : o0 + CH],
                            in_=et[:, jb % 8, i_start:CH],
                        )

            def c2_av(h, c2, h2s=(0, 1)):
                """AV + ones row-sums + normalize for query chunk c2."""
                i0 = CH * c2
                njb = 8 * c2 + 8
                tiles = e8s[(h, c2)]
                e8f = e8s.get(("f", h, c2))
                if 1 in h2s:
                    e8s.pop((h, c2))
                    e8s.pop(("f", h, c2), None)
                for h2 in h2s:
                    c0g, c1g = 512 * h2, 512 * (h2 + 1)
                    u_ps = psp.tile([P, 512], F32, tag="u", bufs=1)
                    r_ps = psp.tile([P, 512], F32, tag="r", bufs=1)
                    last_jb = (8 * c2 + 3) if h2 == 0 else (njb - 1)
                    started = False
                    for jb in range(njb):
                        i_start = max(0, P * jb - i0)
                        s0, s1 = max(c0g, i_start), c1g
                        if s0 >= s1:
                            continue
                        et = tiles[jb // 8]
                        nc.tensor.matmul(
                            u_ps[:, s0 - c0g : s1 - c0g],
                            lhsT=vt[:, jb, P * h : P * (h + 1)],
                            rhs=et[:, jb % 8, s0:s1],
                            start=(not started),
                            stop=(jb == last_jb),
                            skip_group_check=True,
                        )
                        if c2 == 0:
                            nc.tensor.matmul(
                                r_ps[:, s0 - c0g : s1 - c0g],
                                lhsT=onest[:],
                                rhs=et[:, jb % 8, s0:s1],
                                start=(not started),
                                stop=(jb == last_jb),
                                skip_group_check=True,
                            )
                        started = True
                    if c2 == 1:
                        # fp8e5 DoubleRow row-sums: one matmul per key-block
                        # pair at 2 blocks/pass (halves the ones-matmul cost)
                        pairs = [
                            g
                            for g in range(4 * (c2 + 1))
                            if max(c0g, PAIR_IS0[c2][g]) < c1g
                        ]
                        for gi, g in enumerate(pairs):
                            s0 = max(c0g, PAIR_IS0[c2][g])
                            o0 = PAIR_OFF[c2][g] - PAIR_IS0[c2][g]
                            nc.tensor.matmul(
                                r_ps[:, s0 - c0g : c1g - c0g],
                                lhsT=ones8t[:, :, :],
                                rhs=e8f[:, :, o0 + s0 : o0 + c1g],
                                perf_mode=mybir.MatmulPerfMode.DoubleRow,
                                start=(gi == 0),
                                stop=(gi == len(pairs) - 1),
                                skip_group_check=True,
                            )
                    inv_r = cp.tile([P, 512], F32, tag="invr", bufs=2)
                    nc.vector.reciprocal_approx_fast(inv_r[:], r_ps[:])
                    nc.vector.tensor_tensor(
                        oTs[h][:, i0 + c0g : i0 + c1g],
                        u_ps[:],
                        inv_r[:],
                        mybir.AluOpType.mult,
                    )

            def out_proj(woT, its):
                """partial[i, e] = sum_m o[i, m] wo[e, m] for token tiles its."""
                for it in its:
                    for ec in range(NI):
                        ps = psp.tile([P, 512], F32, tag="pj", name="outps")
                        for h in range(NH):
                            nc.tensor.matmul(
                                ps[:],
                                lhsT=oTs[h][:, P * it : P * (it + 1)],
                                rhs=woT[:, h, 512 * ec : 512 * (ec + 1)],
                                start=(h == 0),
                                stop=(h == NH - 1),
                            )
                        ost = osp.tile([P, 512], F16, tag="ostage", bufs=4)
                        if (it * NI + ec) % 2 == 0:
                            nc.vector.tensor_copy(out=ost[:], in_=ps[:])
                        else:
                            nc.scalar.copy(ost[:], ps[:])
                        eng = nc.sync if (it * NI + ec) % 2 == 0 else nc.scalar
                        eng.dma_start(out_r[it][:, 512 * ec : 512 * (ec + 1)], ost[:])

            # ---------------- schedule ----------------
            qk_proj0()
            c1(0, 0)
            c1(0, 1)

            # V[p, it, m] = v[it*128+p, m] = sum_d x[i, d] wv[m, d]
            for it in range(NT):
                ps = psp.tile([P, 512], F32, tag="pj", name="vps")
                for d in range(ND):
                    nc.tensor.matmul(
                        ps[:],
                        lhsT=xT[:, d, P * it : P * (it + 1)],
                        rhs=wvT[:, d, :],
                        start=(d == 0),
                        stop=(d == ND - 1),
                    )
                nc.vector.tensor_copy(out=vt[:, it, :], in_=ps[:])

            c2_av(0, 0)
            c2_av(0, 1)
            for h in range(1, NH):
                qk_proj(h)
                c1(h, 0)
                c1(h, 1)
                c2_av(h, 0)
                if h == NH - 1:
                    # wot reuses xT's slot; its DMA waits xT's last read (QK(3))
                    woT = bigp.tile([P, NH, D], BF16, tag="xT")
                    nc.sync.dma_start(woT[:], wot_r[:, :, :])
                    out_proj(woT, range(0, 8))
                    c2_av(h, 1, h2s=(0,))
                    out_proj(woT, range(8, 12))
                    c2_av(h, 1, h2s=(1,))
                    out_proj(woT, range(12, 16))
                else:
                    c2_av(h, 1)

    nc.compile()
    return nc


def make_in_maps(x, Wq, Wk, Wv, Wo):
    bf = ml_dtypes.bfloat16
    ones_bf = np.ones((P, P), dtype=bf)
    ones8 = np.ones((P, 2, P), dtype=ml_dtypes.float8_e5m2)
    jj, ii = np.meshgrid(np.arange(P), np.arange(P), indexing="ij")
    tri = (jj <= ii).astype(bf)  # tri[j, i] = j <= i

    xtb = [np.ascontiguousarray(x[0].T).astype(bf), np.ascontiguousarray(x[1].T).astype(bf)]
    in_maps = []
    for c in range(8):
        b, hg = c // 4, c % 4
        sl = slice(M * hg, M * (hg + 1))
        in_maps.append(
            {
                "xt": xtb[b],
                "wqt": np.ascontiguousarray(Wq[sl].T).astype(bf),
                "wkt": np.ascontiguousarray(Wk[sl].T).astype(bf),
                "wvt": np.ascontiguousarray(Wv[sl].T).astype(bf),
                "wot": np.ascontiguousarray(Wo[:, sl].T).astype(bf),
                "ones_bf": ones_bf,
                "ones8": ones8,
                "tri": tri,
            }
        )
    return in_maps


def kernel(x, mask, Wq, Wk, Wv, Wo, _trace=False):
    global _CACHED_NC
    x = np.asarray(x, dtype=np.float32)
    Wq = np.asarray(Wq, dtype=np.float32)
    Wk = np.asarray(Wk, dtype=np.float32)
    Wv = np.asarray(Wv, dtype=np.float32)
    Wo = np.asarray(Wo, dtype=np.float32)
    if _CACHED_NC is None:
        _CACHED_NC = build_nc()
    nc = _CACHED_NC
    in_maps = make_in_maps(x, Wq, Wk, Wv, Wo)
    res = run_bass_kernel_spmd(nc, in_maps, list(range(8)), trace=_trace)
    outs = [np.asarray(r["out"], dtype=np.float32) for r in res.results]
    full = np.empty((2, S, D), dtype=np.float32)
    for b in range(2):
        full[b] = outs[4 * b] + outs[4 * b + 1] + outs[4 * b + 2] + outs[4 * b + 3]
    kernel.last_exec_time_ns = res.exec_time_ns
    return full
